# revision 27
# baseline (speedup 1.0000x reference)
"""DETR loss (nn_DetrLoss) on 8 Trainium2 NeuronCores.

Strategy (data-parallel over batch, per the sharding hint):
  - Device (8 cores x 2 batch elements each): compute the 300x300 matching
    cost matrix per batch element (class-score gather + L1 + GIoU, with the
    no-object indicator) plus per-row sum(exp(.)) of pred_class. This is all
    the O(Q^2) / O(Q*C) heavy compute.
  - Host: argmax of the one-hot gt_class (preprocessing), Hungarian matching
    on the device cost matrices (inherently sequential), and the final scalar
    reduction over the 4800 matched pairs (trivial numpy work).

Device layout: cost matrix in [j(gt) partitions, i(pred) free] layout.
The two batch elements a core owns are packed into the partition dim:
partitions 0:64 hold b0 rows, 64:128 hold b1 rows; 300 j's per b -> 5 chunks
(4x64-wide + 1x44-wide per b). Class scores come from an indirect-DMA row
gather of the host-transposed pred_class. gt-side per-j features ride as
per-partition [P,1] scalars; pred-side per-i features are broadcast along
partitions once per core via a stride-0 DMA. The L1/GIoU arithmetic runs as
fused custom-DVE ops (one instruction per ~4-7 ALU stages) with fast-Newton
reciprocals, with a few tensor_tensor ops offloaded to GpSimd.
"""
import os
import numpy as np

B, Q, C1 = 16, 300, 1001
NCORES = 8
BPC = 2            # batch elements per core
NCH = 5            # partition chunks per core (j dim, both b's packed)
CW = 64            # chunk width per batch element (last chunk: 44)
EPS = 1e-7

_NC_CACHE = {}
_LAST_RUN = {}


def _chunk_width(c):
    return CW if c < NCH - 1 else Q - CW * (NCH - 1)  # 64,64,64,64,44


# ---------------------------------------------------------------------------
# custom DVE ops (registered once into concourse.dve_ops.OPS)
# ---------------------------------------------------------------------------

def _register_dve_ops():
    import concourse.dve_ops as dve_ops
    from concourse.dve_ops import DveOp, has_src1
    from concourse.dve_spec import Spec, Src0, Src1, C0, C1 as DC1, C2, One, lower, maxx, minn, relu
    from concourse.dve_uop import DveOpSpec

    if "detr_ops" in _NC_CACHE:
        return _NC_CACHE["detr_ops"]

    def _mk(name, body, ref):
        spec = Spec(body=body, reference=ref)
        row = dve_ops._CUSTOM_DVE_ROW_BASE + len(dve_ops.OPS)
        assert row < 0x20, "custom-DVE opcode rows exhausted"
        shas = {}
        for ver in ("v3",):
            compiled = DveOpSpec(name=name, opcode=row,
                                 uops=lower(spec, ver=ver), rd1_en=has_src1(spec))
            shas[ver] = compiled.sha(ver)
        op = DveOp(name, spec, subdim=False, uops_sha=shas)
        dve_ops.OPS.append(op)
        dve_ops._SUB_OPCODE_FOR_NAME[name] = row
        dve_ops.CUSTOM_DVE_SPECS[name] = spec
        return op

    ops = {}
    # riw = relu(min(px2, gx2) - max(px1, gx1)); Src0=px1, Src1=px2, C0=gx1, C1=gx2
    ops["RIW"] = _mk("DETR_RIW",
                     relu(minn(Src1, DC1) - maxx(Src0, C0)),
                     lambda in0, in1, c0, c1, c2: np.maximum(
                         np.minimum(in1, c1) - np.maximum(in0, c0), 0.0))
    # cw = max(px2, gx2) - min(px1, gx1)
    ops["CWX"] = _mk("DETR_CW",
                     maxx(Src1, DC1) - minn(Src0, C0),
                     lambda in0, in1, c0, c1, c2: np.maximum(in1, c1) - np.minimum(in0, c0))
    # l1 pair: |pcx-gcx| + |pcy-gcy|
    ops["ABS2"] = _mk("DETR_ABS2",
                      maxx(Src0 - C0, C0 - Src0) + maxx(Src1 - DC1, DC1 - Src1),
                      lambda in0, in1, c0, c1, c2: np.abs(in0 - c0) + np.abs(in1 - c1))
    # union+eps = (parea + garea_eps) - inter ; Src0=parea, Src1=inter, C0=garea+eps
    ops["UNIONP"] = _mk("DETR_UNIONP",
                        (Src0 + C0) - Src1,
                        lambda in0, in1, c0, c1, c2: (in0 + c0) - in1)
    # areac+eps = cw*ch + eps(imm)
    ops["MULEPS"] = _mk("DETR_MULEPS",
                        Src0 * Src1 + C2,
                        lambda in0, in1, c0, c1, c2: in0 * in1 + c2)
    # z1 = cls + l1 + 1
    ops["ADD1"] = _mk("DETR_ADD1",
                      (Src0 + Src1) + One,
                      lambda in0, in1, c0, c1, c2: (in0 + in1) + 1.0)
    # cost = (z1 + fmi) * ind
    ops["SUMSCALE"] = _mk("DETR_SUMSCALE",
                          (Src0 + Src1) * C0,
                          lambda in0, in1, c0, c1, c2: (in0 + in1) * c0)
    # l1 pair + 1 (carries the "+1" of the cost through the PSUM accumulate)
    ops["ABS2P"] = _mk("DETR_ABS2P",
                       maxx(Src0 - C0, C0 - Src0) + maxx(Src1 - DC1, DC1 - Src1) + One,
                       lambda in0, in1, c0, c1, c2: np.abs(in0 - c0) + np.abs(in1 - c1) + 1.0)
    # negated union+eps = inter - (parea + garea_eps)
    ops["UNIONN"] = _mk("DETR_UNIONN",
                        Src1 - (Src0 + C0),
                        lambda in0, in1, c0, c1, c2: in1 - (in0 + c0))
    # frac + 1 = 1 + (-(union+eps)) * (1/(areac+eps))
    ops["FRAC1"] = _mk("DETR_FRAC1",
                       Src0 * Src1 + One,
                       lambda in0, in1, c0, c1, c2: in0 * in1 + 1.0)
    _NC_CACHE["detr_ops"] = ops
    return ops


def _build_nc(use_gather=True, use_bcast=True, use_cost=True, use_lse=True):
    import concourse.bacc as bacc
    import concourse.tile as tile
    import concourse.mybir as mybir
    import concourse.bass as bass
    from contextlib import ExitStack

    F32 = mybir.dt.float32
    I32 = mybir.dt.int32
    Alu = mybir.AluOpType
    Act = mybir.ActivationFunctionType
    OPS = _register_dve_ops()

    nc = bacc.Bacc("TRN2", target_bir_lowering=False, debug=False)

    pcT = nc.dram_tensor("pcT", [BPC * C1, Q], F32, kind="ExternalInput")
    pc = nc.dram_tensor("pc", [BPC, Q, C1], F32, kind="ExternalInput")
    prows = nc.dram_tensor("prows", [BPC, 9 * Q], F32, kind="ExternalInput")
    idm = nc.dram_tensor("idm", [128, 128], F32, kind="ExternalInput")
    gtall = nc.dram_tensor("gtall", [128, NCH * 11], F32, kind="ExternalInput")
    ct = nc.dram_tensor("ct", [NCH, 128, Q], F32, kind="ExternalOutput")
    esum = nc.dram_tensor("esum", [BPC, Q, 1], F32, kind="ExternalOutput")

    with tile.TileContext(nc) as tc, ExitStack() as ctx:
        rep_pool = ctx.enter_context(tc.tile_pool(name="rep", bufs=1))
        io_pool = ctx.enter_context(tc.tile_pool(name="io", bufs=1))
        w_pool = ctx.enter_context(tc.tile_pool(name="w", bufs=1))
        psum_pool = ctx.enter_context(tc.tile_pool(name="ps", bufs=1, space="PSUM"))
        ls_pool = ctx.enter_context(tc.tile_pool(name="ls", bufs=2))

        # Replicated pred-side features [128, 9*Q]: partitions 0:64 <- b0's
        # rows, 64:128 <- b1's rows. Replicated on-chip: one tiny DMA brings
        # prows to SBUF, PE outer-products it across partitions via a [2,128]
        # selector (exact 0/1 weights), ACT copies PSUM->SBUF. Avoids a
        # 2.8MB broadcast DMA that otherwise stalls the pipeline start.
        # one packed DMA carries all 5 chunks' gt features + gather indices
        gtall_t = io_pool.tile([128, NCH * 11], F32, name="gtall_t")
        nc.sync.dma_start(out=gtall_t[:], in_=gtall[:, :])

        # Two replicated-feature tiles so the GIoU ops (which only need
        # xyxy+parea) start as soon as the first-half broadcast lands,
        # without waiting for the cxcywh half.
        repA = rep_pool.tile([128, 5 * Q], F32, name="repA")
        repB = rep_pool.tile([128, 4 * Q], F32, name="repB")
        if use_bcast:
            # stride-0 partition-broadcast DMAs spread across three DGE queues
            h = 5 * Q
            nc.sync.dma_start(out=repA[0:64, :], in_=prows[0:1, :h].to_broadcast([64, h]))
            nc.scalar.dma_start(out=repA[64:128, :], in_=prows[1:2, :h].to_broadcast([64, h]))
            nc.gpsimd.dma_start(out=repB[0:64, :], in_=prows[0:1, h:].to_broadcast([64, 4 * Q]))
            nc.gpsimd.dma_start(out=repB[64:128, :], in_=prows[1:2, h:].to_broadcast([64, 4 * Q]))
        else:
            nc.vector.memset(repA[:], 0.25)
            nc.vector.memset(repB[:], 0.25)
        PX1, PY1, PX2, PY2, PAREA, PCX, PCY, PW, PH = range(9)

        def R(r):
            return repA[:, r * Q:(r + 1) * Q] if r < 5 else repB[:, (r - 5) * Q:(r - 4) * Q]

        V = nc.vector
        G = nc.gpsimd
        A = nc.scalar

        # Wavefront emission: level-by-level across all 5 chunks, so each
        # engine sees 5 independent instructions per dependency level and
        # cross-engine handoff latency is amortized.
        GX1, GY1, GX2, GY2, GCX, GCY, GW, GH, GAREA_EPS, GIND, GIDX = range(11)
        NC_ = NCH if use_cost else 0
        cd = V._custom_dve
        st = {}

        def wt(tag, c, width=Q):
            return w_pool.tile([128, width], F32, tag=f"{tag}{c}", name=f"{tag}{c}")

        def S_(c, k):
            return gtall_t[:, c * 11 + k:c * 11 + k + 1]

        for c in range(NC_):
            st[c] = {}
            cls_t = wt("cls", c)
            if not use_gather:
                V.memset(cls_t[:], 0.5)
            else:
                G.indirect_dma_start(
                    out=cls_t[:], out_offset=None, in_=pcT[:, :],
                    in_offset=bass.IndirectOffsetOnAxis(
                        ap=S_(c, GIDX).bitcast(I32), axis=0),
                )
            st[c]["cls"] = cls_t

        # ---- sum(exp(x)) over pred_class rows; host takes the log.
        # Emitted early so ACT's big Exp ops overlap the cost pipeline.
        for b in range(BPC if use_lse else 0):
            for r0 in (0, 128, 256):
                p = min(128, Q - r0)
                pc_t = ls_pool.tile([128, C1], F32, tag="pc_t")
                G.dma_start(out=pc_t[:p, :], in_=pc[b, r0:r0 + p, :])
                ex_t = ls_pool.tile([128, C1], F32, tag="ex_t")
                s_t = ls_pool.tile([128, 1], F32, tag="s_t")
                A.activation(ex_t[:p, :], pc_t[:p, :], Act.Exp, accum_out=s_t[:p, :])
                A.dma_start(out=esum[b, r0:r0 + p, :], in_=s_t[:p, :])

        # identity for PE copy-accumulate matmuls (host-supplied; building it
        # on GpSimd would trigger the ~10us Q7 library-load preamble)
        ident = rep_pool.tile([128, 128], F32, name="ident")
        nc.sync.dma_start(out=ident[:], in_=idm[:, :])
        PE = nc.tensor

        for c in range(NC_):
            s = st[c]
            S = lambda k: S_(c, k)
            s["riw"] = wt("riw", c); cd(OPS["RIW"], out=s["riw"][:], in0=R(PX1), in1=R(PX2), s0=S(GX1), s1=S(GX2))
            s["rih"] = wt("rih", c); cd(OPS["RIW"], out=s["rih"][:], in0=R(PY1), in1=R(PY2), s0=S(GY1), s1=S(GY2))
            s["cw"] = wt("cw", c); cd(OPS["CWX"], out=s["cw"][:], in0=R(PX1), in1=R(PX2), s0=S(GX1), s1=S(GX2))
            s["ch"] = wt("ch", c); cd(OPS["CWX"], out=s["ch"][:], in0=R(PY1), in1=R(PY2), s0=S(GY1), s1=S(GY2))
            s["l1a"] = wt("l1a", c); cd(OPS["ABS2"], out=s["l1a"][:], in0=R(PCX), in1=R(PCY), s0=S(GCX), s1=S(GCY))
            s["l1b"] = wt("l1b", c); cd(OPS["ABS2"], out=s["l1b"][:], in0=R(PW), in1=R(PH), s0=S(GW), s1=S(GH))
        # PSUM accumulators for the early terms: cls + (l1a+1) + l1b
        for c in range(NC_):
            s = st[c]
            s["acc"] = psum_pool.tile([128, Q], F32, space="PSUM", name=f"acc{c}")
            PE.matmul(out=s["acc"][:], lhsT=ident[:], rhs=s["cls"][:], start=True, stop=False)
            PE.matmul(out=s["acc"][:], lhsT=ident[:], rhs=s["l1a"][:], start=False, stop=False)
            PE.matmul(out=s["acc"][:], lhsT=ident[:], rhs=s["l1b"][:], start=False, stop=True)
        for c in range(NC_):
            s = st[c]
            s["inter"] = wt("inter", c)
            V.tensor_tensor(out=s["inter"][:], in0=s["riw"][:], in1=s["rih"][:], op=Alu.mult)
        for c in range(NC_):
            s = st[c]
            S = lambda k: S_(c, k)
            s["ua"] = wt("ua", c, 2 * Q)
            cd(OPS["MULEPS"], out=s["ua"][:, Q:2 * Q], in0=s["cw"][:], in1=s["ch"][:], imm2=float(EPS))
            # negated union: ua[0:Q] = inter - (parea + garea_eps) = -(union+eps)
            cd(OPS["UNIONN"], out=s["ua"][:, 0:Q], in0=R(PAREA), in1=s["inter"][:], s0=S(GAREA_EPS))
        for c in range(NC_):
            s = st[c]
            s["rr"] = wt("rr", c, 2 * Q)
            V.reciprocal_approx_fast(out=s["rr"][:], in_=s["ua"][:])
        for c in range(NC_):
            s = st[c]
            # -iou = inter * (-1/U')
            s["ioun"] = wt("ioun", c)
            V.tensor_tensor(out=s["ioun"][:], in0=s["inter"][:], in1=s["rr"][:, 0:Q], op=Alu.mult)
            # frac + 1 = 1 + nU * (1/A')
            s["frac1"] = wt("frac1", c)
            cd(OPS["FRAC1"], out=s["frac1"][:], in0=s["ua"][:, 0:Q], in1=s["rr"][:, Q:2 * Q])
        for c in range(NC_):
            s = st[c]
            s["fmi"] = wt("fmi", c)
            V.tensor_tensor(out=s["fmi"][:], in0=s["frac1"][:], in1=s["ioun"][:], op=Alu.add)
        for c in range(NC_):
            s = st[c]
            S = lambda k: S_(c, k)
            s["cost"] = wt("cost", c)
            cd(OPS["SUMSCALE"], out=s["cost"][:], in0=s["acc"][:], in1=s["fmi"][:], s0=S(GIND))
            G.dma_start(out=ct[c], in_=s["cost"][:])

    nc.compile()
    return nc


def _prep_host(pred_class, pred_boxes, gt_class, gt_boxes):
    f = np.float32
    ga = np.argmax(gt_class, axis=-1).astype(np.int32)          # [B,Q]
    ind = (f(1.0) - gt_class[:, :, -1]).astype(f)               # [B,Q]

    pcx, pcy, pw, ph = (pred_boxes[..., k] for k in range(4))
    gcx, gcy, gw, gh = (gt_boxes[..., k] for k in range(4))
    px1, py1 = (pcx - pw / 2).astype(f), (pcy - ph / 2).astype(f)
    px2, py2 = (pcx + pw / 2).astype(f), (pcy + ph / 2).astype(f)
    gx1, gy1 = (gcx - gw / 2).astype(f), (gcy - gh / 2).astype(f)
    gx2, gy2 = (gcx + gw / 2).astype(f), (gcy + gh / 2).astype(f)
    parea = ((px2 - px1) * (py2 - py1)).astype(f)
    garea_eps = ((gx2 - gx1) * (gy2 - gy1) + f(EPS)).astype(f)

    prows_all = np.stack([px1, py1, px2, py2, parea, pcx, pcy, pw, ph], axis=1)  # [B,9,Q]
    gtsc_all = np.stack([gx1, gy1, gx2, gy2, gcx, gcy, gw, gh, garea_eps, ind], axis=-1)
    pcT_all = np.ascontiguousarray(pred_class.transpose(0, 2, 1))  # [B,C1,Q]

    in_maps = []
    for k in range(NCORES):
        b0 = BPC * k
        # packed per-partition gt table: [128, NCH*11]; 10 features + the
        # gather index (int32 bit-cast into the f32 slot)
        gtall_k = np.zeros((128, NCH, 11), f)
        for c in range(NCH):
            w = _chunk_width(c)
            j0 = CW * c
            for lb in range(BPC):
                gtall_k[64 * lb:64 * lb + w, c, :10] = gtsc_all[b0 + lb, j0:j0 + w]
                gtall_k[64 * lb:64 * lb + w, c, 10] = (
                    (lb * C1 + ga[b0 + lb, j0:j0 + w]).astype(np.int32).view(f))
        idm = np.eye(128, dtype=f)
        in_maps.append({
            "pcT": pcT_all[b0:b0 + BPC].reshape(BPC * C1, Q).copy(),
            "pc": pred_class[b0:b0 + BPC].copy(),
            "prows": prows_all[b0:b0 + BPC].reshape(BPC, 9 * Q).copy(),
            "gtall": gtall_k.reshape(128, NCH * 11),
            "idm": idm,
        })
    return in_maps, ga


def _fake_device(in_maps):
    """Numpy emulation of the device kernel (same packing), for offline tests."""
    f = np.float32
    outs = []
    for m in in_maps:
        pcT, prows = m["pcT"], m["prows"]
        gtall = m["gtall"].reshape(128, NCH, 11)
        gtsc = np.transpose(gtall[:, :, :10], (1, 0, 2))
        gidx = np.transpose(gtall[:, :, 10:11], (1, 0, 2)).view(np.int32)
        rep = np.zeros((128, 9 * Q), f)
        rep[0:64] = prows[0]
        rep[64:128] = prows[1]
        Rv = lambda r: rep[:, r * Q:(r + 1) * Q]
        PX1, PY1, PX2, PY2, PAREA, PCX, PCY, PW, PH = range(9)
        ct_o = np.zeros((NCH, 128, Q), f)
        for c in range(NCH):
            Sv = lambda k: gtsc[c][:, k:k + 1]
            GX1, GY1, GX2, GY2, GCX, GCY, GW, GH, GAREA_EPS, GIND = range(10)
            cls = pcT[gidx[c, :, 0]]
            l1 = (np.abs(Rv(PCX) - Sv(GCX)) + np.abs(Rv(PCY) - Sv(GCY))
                  + np.abs(Rv(PW) - Sv(GW)) + np.abs(Rv(PH) - Sv(GH))).astype(f)
            riw = np.maximum(np.minimum(Rv(PX2), Sv(GX2)) - np.maximum(Rv(PX1), Sv(GX1)), 0)
            rih = np.maximum(np.minimum(Rv(PY2), Sv(GY2)) - np.maximum(Rv(PY1), Sv(GY1)), 0)
            inter = (riw * rih).astype(f)
            unio = ((Rv(PAREA) + Sv(GAREA_EPS)) - inter).astype(f)
            cw = np.maximum(Rv(PX2), Sv(GX2)) - np.minimum(Rv(PX1), Sv(GX1))
            chh = np.maximum(Rv(PY2), Sv(GY2)) - np.minimum(Rv(PY1), Sv(GY1))
            aeps = (cw * chh + f(EPS)).astype(f)
            iou = (inter / unio).astype(f)
            frac = ((aeps - unio) / aeps).astype(f)
            acc = cls + (np.abs(Rv(PCX) - Sv(GCX)) + np.abs(Rv(PCY) - Sv(GCY)) + f(1.0))
            acc = acc + (np.abs(Rv(PW) - Sv(GW)) + np.abs(Rv(PH) - Sv(GH)))
            acc = acc - iou + frac
            ct_o[c] = acc * Sv(GIND)
        pc = m["pc"]
        es_o = np.exp(pc.astype(f)).sum(-1, dtype=f).astype(f)[..., None]
        outs.append({"ct": ct_o, "esum": es_o})
    return outs


def _unpack(results):
    cost = np.empty((B, Q, Q), np.float32)   # [b, i, j]
    esum = np.empty((B, Q), np.float32)
    for k in range(NCORES):
        b0 = BPC * k
        ctk = results[k]["ct"]
        for lb in range(BPC):
            ct_b = np.empty((Q, Q), np.float32)
            for c in range(NCH):
                w = _chunk_width(c)
                ct_b[CW * c:CW * c + w] = ctk[c, 64 * lb:64 * lb + w]
            cost[b0 + lb] = ct_b.T
            esum[b0 + lb] = results[k]["esum"][lb, :, 0]
    return cost, np.log(esum.astype(np.float64))


def _lsa(cost):
    # Hungarian (shortest augmenting path, e-maxx) — mirrors the reference.
    n = cost.shape[0]
    INF = np.inf
    u = np.zeros(n + 1)
    v = np.zeros(n + 1)
    p = np.zeros(n + 1, dtype=np.int64)
    way = np.zeros(n + 1, dtype=np.int64)
    for i in range(1, n + 1):
        p[0] = i
        j0 = 0
        minv = np.full(n + 1, INF)
        used = np.zeros(n + 1, dtype=bool)
        while True:
            used[j0] = True
            i0 = p[j0]
            cur = cost[i0 - 1, :] - u[i0] - v[1:]
            cur = np.where(used[1:], INF, cur)
            upd = cur < minv[1:]
            minv[1:] = np.where(upd, cur, minv[1:])
            way[1:] = np.where(upd, j0, way[1:])
            masked = np.where(used[1:], INF, minv[1:])
            j1 = int(np.argmin(masked)) + 1
            delta = masked[j1 - 1]
            u[p[used]] += delta
            v[used] -= delta
            minv[1:] = np.where(used[1:], minv[1:], minv[1:] - delta)
            j0 = j1
            if p[j0] == 0:
                break
        while j0 != 0:
            j1 = way[j0]
            p[j0] = p[j1]
            j0 = j1
    col_for_row = np.empty(n, dtype=np.int64)
    col_for_row[p[1:] - 1] = np.arange(n)
    return col_for_row


def _final_loss(pred_class, pred_boxes, gt_boxes, ga, gt_idx, lse):
    lab = np.take_along_axis(ga.astype(np.int64), gt_idx, axis=1)
    x_lab = np.take_along_axis(pred_class, lab[:, :, None], axis=2)[..., 0]
    ce = (lse - x_lab.astype(np.float64)).mean(1).sum()
    gtb = np.take_along_axis(gt_boxes, gt_idx[:, :, None], axis=1).astype(np.float64)
    pb = pred_boxes.astype(np.float64)
    l1 = np.abs(pb - gtb).sum()

    def xyxy(bx):
        return np.concatenate([bx[..., :2] - bx[..., 2:] / 2,
                               bx[..., :2] + bx[..., 2:] / 2], -1)
    b1, b2 = xyxy(pb), xyxy(gtb)
    x1 = np.maximum(b1[..., 0], b2[..., 0]); y1 = np.maximum(b1[..., 1], b2[..., 1])
    x2 = np.minimum(b1[..., 2], b2[..., 2]); y2 = np.minimum(b1[..., 3], b2[..., 3])
    inter = np.maximum(x2 - x1, 0) * np.maximum(y2 - y1, 0)
    a1 = (b1[..., 2] - b1[..., 0]) * (b1[..., 3] - b1[..., 1])
    a2 = (b2[..., 2] - b2[..., 0]) * (b2[..., 3] - b2[..., 1])
    union = a1 + a2 - inter
    iou = inter / (union + EPS)
    cx1 = np.minimum(b1[..., 0], b2[..., 0]); cy1 = np.minimum(b1[..., 1], b2[..., 1])
    cx2 = np.maximum(b1[..., 2], b2[..., 2]); cy2 = np.maximum(b1[..., 3], b2[..., 3])
    area_c = (cx2 - cx1) * (cy2 - cy1)
    giou = iou - (area_c - union) / (area_c + EPS)
    giou_loss = (1.0 - giou).sum()
    return ce + l1 + giou_loss


def kernel(pred_class, pred_boxes, gt_class, gt_boxes):
    pred_class = np.ascontiguousarray(pred_class, dtype=np.float32)
    pred_boxes = np.ascontiguousarray(pred_boxes, dtype=np.float32)
    gt_class = np.ascontiguousarray(gt_class, dtype=np.float32)
    gt_boxes = np.ascontiguousarray(gt_boxes, dtype=np.float32)

    in_maps, ga = _prep_host(pred_class, pred_boxes, gt_class, gt_boxes)

    if os.environ.get("DETR_FAKE_DEVICE"):
        results = _fake_device(in_maps)
        _LAST_RUN["res"] = None
    else:
        from concourse.bass_utils import run_bass_kernel_spmd
        if "nc" not in _NC_CACHE:
            _NC_CACHE["nc"] = _build_nc()
        res = run_bass_kernel_spmd(
            _NC_CACHE["nc"], in_maps, core_ids=list(range(NCORES)),
            trace=bool(os.environ.get("DETR_TRACE")),
        )
        results = res.results
        _LAST_RUN["res"] = res

    cost, lse = _unpack(results)
    gt_idx = np.stack([_lsa(cost[b]) for b in range(B)], axis=0)
    total = _final_loss(pred_class, pred_boxes, gt_boxes, ga, gt_idx, lse)
    return np.asarray(total, dtype=np.float32)
